# revision 21
# baseline (speedup 1.0000x reference)
"""GQA attention (B=2, S=2048, D=2048, H=32, G=8, hd=64) on 8 TRN2 cores.

ZERO-COLLECTIVE sharding: core c owns (batch b=c//4, token block
q0=512*(c%4)). Each core computes the FULL output slice out[b,
q0:q0+512, :] independently: full K/V over all S (replicated within a
batch group), Q only for its own 512 tokens, attention + output
projection fully local. Collectives are deliberately avoided: on this
runtime an AllGather trips a GPIO power throttle that caps the PE clock
at 81% for most of the kernel (~+100us) on top of ~60-100us latency.

Per-core SPMD uniformity: the token axis of x^T (and the RoPE tables) is
ROTATED by q0 on the host, so "own tokens" are always columns 0:512.
Attention is permutation-invariant over keys; RoPE phases ride with the
rotation.

Schedule: x^T streams in 512-token chunks; attention round 0 pair 0
rides the first pass chunk-by-chunk (K tiles 0,1 + V + Q per chunk), so
the exp stream starts at ~40us instead of waiting for the full 8.4 MB
x^T load. K tiles 2,3 and remaining Q chains interleave into later
pairs' PE slack, keeping the PE dense (HAM clock stays 8/8). Softmax
denominators accumulate free in PSUM row 64 via an augmented-V ones
column; reciprocals run on DVE (reciprocal_approx_fast) so ACT does
pure exp with a single table load (preheated).

PSUM: scores 2x[128,1024] (4 banks) + chain accumulators 2x[128,512]
(2) + PV accumulator [65,1024] (2, heads A|B side by side). The output
projection reuses all 8 banks as 8 held accumulators while wo streams
through SBUF in 0.5 MB chunks.
"""

import sys

sys.path.insert(0, "/opt/trn_rl_repo")

import numpy as np
import ml_dtypes

import concourse.bass as bass
import concourse.tile as tile
from concourse import bacc, mybir
from concourse.bass_utils import run_bass_kernel_spmd

BF16 = ml_dtypes.bfloat16
B, S, D = 2, 2048, 2048
H, G, HD = 32, 8, 64
DC = D // 128  # 16 dim chunks
N_CORES = 8
TOK = 512  # own tokens per core

_CACHE = {}


def _build():
    f32 = mybir.dt.float32
    bf16 = mybir.dt.bfloat16
    nc = bacc.Bacc("TRN2", target_bir_lowering=False, debug=False, num_devices=N_CORES)

    xt = nc.dram_tensor("xt", [128, DC, S], bf16, kind="ExternalInput").ap()
    wq = nc.dram_tensor("wq", [128, DC, DC, 128], bf16, kind="ExternalInput").ap()
    wk = nc.dram_tensor("wk", [128, DC, 4, 128], bf16, kind="ExternalInput").ap()
    wv = nc.dram_tensor("wv", [128, DC, 512], bf16, kind="ExternalInput").ap()
    cosr = nc.dram_tensor("cosr", [128, S], bf16, kind="ExternalInput").ap()
    sinr = nc.dram_tensor("sinr", [128, S], bf16, kind="ExternalInput").ap()
    wo = nc.dram_tensor("wo", [128, DC, D], bf16, kind="ExternalInput").ap()
    out = nc.dram_tensor("out", [TOK, D], f32, kind="ExternalOutput").ap()

    Exp = mybir.ActivationFunctionType.Exp
    swap_mask = [i ^ 1 for i in range(32)]
    scale = float(1.0 / np.sqrt(HD))

    from contextlib import ExitStack
    with tile.TileContext(nc) as tc, ExitStack() as ctx:
        consts = ctx.enter_context(tc.tile_pool(name="consts", bufs=1))
        xtfp = ctx.enter_context(tc.tile_pool(name="xtfp", bufs=2))
        wqp = ctx.enter_context(tc.tile_pool(name="wqp", bufs=2))
        io = ctx.enter_context(tc.tile_pool(name="io", bufs=2))
        work = ctx.enter_context(tc.tile_pool(name="work", bufs=3))
        outw = ctx.enter_context(tc.tile_pool(name="outw", bufs=2))
        psum = ctx.enter_context(tc.tile_pool(name="psum", bufs=2, space="PSUM"))
        opsum = ctx.enter_context(tc.tile_pool(name="opsum", bufs=2, space="PSUM"))
        apsum = ctx.enter_context(tc.tile_pool(name="apsum", bufs=1, space="PSUM"))
        dram = ctx.enter_context(tc.tile_pool(name="dram", bufs=1, space="DRAM"))

        # ---- header loads, ordered to unlock K-chunk0 -> Q0 -> V-chunk0.
        # xf0 (own tokens) is persistent: all 16 Q chains read it.
        wk_sb = consts.tile([128, DC, 4, 128], bf16, tag="wk")
        nc.sync.dma_start(out=wk_sb[:, :, 0:1, :], in_=wk[:, :, 0:1, :])
        xf0 = consts.tile([128, DC, 512], bf16, tag="xf0")
        nc.sync.dma_start(out=xf0[:], in_=xt[:, :, 0:512])
        nc.sync.dma_start(out=wk_sb[:, :, 1:2, :], in_=wk[:, :, 1:2, :])
        cosf_sb = consts.tile([128, S], bf16, tag="cosf")
        nc.sync.dma_start(out=cosf_sb[:], in_=cosr[:])
        sinf_sb = consts.tile([128, S], bf16, tag="sinf")
        nc.sync.dma_start(out=sinf_sb[:], in_=sinr[:])
        # wq0 is emitted by q_chain(0) below -> lands here in queue order
        wv_sb = consts.tile([128, DC, 512], bf16, tag="wv")

        # preheat the exp table set (one ACT_TABLE_LOAD, early)
        preheat = io.tile([1, 64], bf16, tag="pre")
        nc.vector.memset(preheat[:], 0.0)
        nc.scalar.activation(preheat[:], preheat[:], Exp)

        kt_sb = consts.tile([128, 4, S], bf16, tag="kt")
        vaug_sb = consts.tile([128, DC, 520], bf16, tag="vaug")
        qt_sb = consts.tile([128, DC, TOK], bf16, tag="qt")
        ot_sb = consts.tile([128, DC, TOK], bf16, tag="ot")
        dstage = consts.tile([97, 4096], f32, tag="dstage")
        nc.vector.memset(dstage[:], 1.0)
        ddram = dram.tile([4, 4096], f32, tag="dd", name="dd")
        # ones columns of augmented V (denominator accumulators)
        for col in (64, 129, 194, 259, 324, 389, 454, 519):
            nc.vector.memset(vaug_sb[:, :, col:col + 1], 1.0)

        def rope(ap, cs, sn):
            sw = io.tile([128, 512], bf16, tag="rsw")
            nc.vector.stream_shuffle(sw, ap, swap_mask)
            nc.vector.tensor_mul(sw, sw, sn)
            tmp = io.tile([128, 512], bf16, tag="rtmp")
            nc.vector.tensor_mul(tmp, ap, cs)
            nc.vector.tensor_add(ap, sw, tmp)

        def xf_dma(ssl, name):
            xf = xtfp.tile([128, DC, 512], bf16, tag="xf", name=name)
            nc.sync.dma_start(out=xf[:], in_=xt[:, :, 512 * ssl:512 * (ssl + 1)])
            return xf

        def pass_K2(ssl, xf):
            # K tiles 0 and 1 interleaved: alternating PSUM banks let the
            # writeback of one chain overlap the other's stream
            sl = slice(512 * ssl, 512 * (ssl + 1))
            psA = opsum.tile([128, 512], f32, tag="o", name=f"ps2a_{ssl}")
            psB = opsum.tile([128, 512], f32, tag="o", name=f"ps2b_{ssl}")
            for c in range(DC):
                nc.tensor.matmul(
                    psA, lhsT=wk_sb[:, c, 0, :], rhs=xf[:, c, :],
                    start=(c == 0), stop=(c == DC - 1),
                )
                nc.tensor.matmul(
                    psB, lhsT=wk_sb[:, c, 1, :], rhs=xf[:, c, :],
                    start=(c == 0), stop=(c == DC - 1),
                )
            for t, ps in ((0, psA), (1, psB)):
                nc.vector.tensor_copy(kt_sb[:, t, sl], ps)
                rope(kt_sb[:, t, sl], cosf_sb[:, sl], sinf_sb[:, sl])

        def pass_K(t, ssl, xf):
            sl = slice(512 * ssl, 512 * (ssl + 1))
            ps = opsum.tile([128, 512], f32, tag="o", name=f"psk{t}_{ssl}")
            for c in range(DC):
                nc.tensor.matmul(
                    ps, lhsT=wk_sb[:, c, t, :], rhs=xf[:, c, :],
                    start=(c == 0), stop=(c == DC - 1),
                )
            nc.vector.tensor_copy(kt_sb[:, t, sl], ps)
            rope(kt_sb[:, t, sl], cosf_sb[:, sl], sinf_sb[:, sl])

        def pass_V(ssl, xf):
            # full V projection for 512 tokens -> vaug cols 0:520
            for tbl in range(4):
                tb = 4 * ssl + tbl
                ps = opsum.tile([128, 512], f32, tag="o", name=f"psv{tb}")
                for c in range(DC):
                    nc.tensor.matmul(
                        ps,
                        lhsT=xf[:, c, tbl * 128:(tbl + 1) * 128],
                        rhs=wv_sb[:, c, :],
                        start=(c == 0), stop=(c == DC - 1),
                    )
                for tt in range(4):
                    base = 130 * tt
                    nc.vector.tensor_copy(
                        vaug_sb[:, tb, base:base + 64],
                        ps[:, 128 * tt:128 * tt + 64])
                    nc.vector.tensor_copy(
                        vaug_sb[:, tb, base + 65:base + 129],
                        ps[:, 128 * tt + 64:128 * tt + 128])

        def q_chain(fc):
            wq_t = wqp.tile([128, DC, 128], bf16, tag="wq", name=f"wqt{fc}")
            nc.sync.dma_start(out=wq_t[:], in_=wq[:, fc, :, :])
            ps = opsum.tile([128, 512], f32, tag="o", name=f"psq{fc}")
            for c in range(DC):
                nc.tensor.matmul(
                    ps, lhsT=wq_t[:, c, :], rhs=xf0[:, c, :],
                    start=(c == 0), stop=(c == DC - 1),
                )
            nc.vector.tensor_copy(qt_sb[:, fc, :], ps)
            rope(qt_sb[:, fc, :], cosf_sb[:, 0:512], sinf_sb[:, 0:512])

        def attn_kb(t, fc, kb0, kb1, po):
            for kb in range(kb0, kb1):
                ksl = slice(kb * 128, (kb + 1) * 128)
                s = psum.tile([128, 1024], f32, tag="s", name=f"s{fc}_{kb}")
                nc.tensor.matmul(
                    s[:, 0:512], lhsT=kt_sb[0:64, t, ksl],
                    rhs=qt_sb[0:64, fc, :],
                    start=True, stop=True, tile_position=(0, 0),
                )
                nc.tensor.matmul(
                    s[:, 512:1024], lhsT=kt_sb[64:128, t, ksl],
                    rhs=qt_sb[64:128, fc, :],
                    start=True, stop=True, tile_position=(64, 0),
                )
                p = work.tile([128, 1024], bf16, tag="p", name=f"p{fc}_{kb}")
                nc.scalar.activation(p, s, Exp, scale=scale)
                nc.tensor.matmul(
                    po[:, 0:512], lhsT=vaug_sb[:, kb, 130 * t:130 * t + 65],
                    rhs=p[:, 0:512],
                    start=(kb == 0), stop=(kb == DC - 1),
                )
                nc.tensor.matmul(
                    po[:, 512:1024], lhsT=vaug_sb[:, kb, 130 * t + 65:130 * t + 130],
                    rhs=p[:, 512:1024],
                    start=(kb == 0), stop=(kb == DC - 1),
                )

        def attn_finish(fc, po):
            nc.vector.tensor_copy(ot_sb[0:64, fc, :], po[0:64, 0:512])
            nc.vector.tensor_copy(ot_sb[64:128, fc, :], po[0:64, 512:1024])
            dp = 32 * (fc % 4)
            df = (fc // 4) * 1024
            nc.vector.tensor_copy(dstage[dp:dp + 1, df:df + 1024],
                                  po[64:65, 0:1024])

        def attn_pair(t, r, interleave=()):
            fc = 4 * t + r
            po = apsum.tile([65, 1024], f32, tag="po", name=f"po{fc}")
            points = sorted(set(k for k, _ in interleave))
            cuts = [0] + points + [DC]
            for i in range(len(cuts) - 1):
                if i > 0:
                    for k, fn in interleave:
                        if k == cuts[i]:
                            fn()
                attn_kb(t, fc, cuts[i], cuts[i + 1], po)
            attn_finish(fc, po)

        def recip_core(t):
            hs = slice(1024 * t, 1024 * (t + 1))
            nc.vector.reciprocal_approx_fast(out=dstage[:, hs], in_=dstage[:, hs])
            for rr in range(4):
                nc.sync.dma_start(out=ddram[rr:rr + 1, hs],
                                  in_=dstage[32 * rr:32 * rr + 1, hs])

        def mul_fc(fc):
            # normalize ot chunk fc by 1/d (broadcast via DRAM bounce rows);
            # spread across later pairs so DVE never spikes at round edges
            def fn():
                dp = fc % 4
                df = (fc // 4) * 1024
                r2 = io.tile([128, TOK], bf16, tag="r2", name=f"r2_{fc}")
                nc.gpsimd.dma_start(
                    out=r2[0:64, :],
                    in_=ddram[dp:dp + 1, df:df + 512].partition_broadcast(64))
                nc.gpsimd.dma_start(
                    out=r2[64:128, :],
                    in_=ddram[dp:dp + 1, df + 512:df + 1024].partition_broadcast(64))
                nc.vector.tensor_mul(ot_sb[:, fc, :], ot_sb[:, fc, :], r2)
            return fn

        def kpass(t, ssl):
            def fn():
                xf = xf_dma(ssl, f"xfk{t}_{ssl}")
                pass_K(t, ssl, xf)
            return fn

        def qc(fc):
            return lambda: q_chain(fc)

        # ================= schedule =================
        # round 0 pair 0 rides the first streaming pass chunk by chunk:
        # per 512-token chunk: K tiles 0,1 + V + one Q chain + 4 kb of
        # attention. Own tokens are chunk 0 (rotated layout).
        po0 = apsum.tile([65, 1024], f32, tag="po", name="po0")

        pass_K2(0, xf0)
        q_chain(0)
        nc.sync.dma_start(out=wv_sb[:], in_=wv[:])  # queued after wq0
        pass_V(0, xf0)
        attn_kb(0, 0, 0, 4, po0)

        for ssl in (1, 2, 3):
            xf = xf_dma(ssl, f"xfa{ssl}")
            pass_K2(ssl, xf)
            pass_V(ssl, xf)
            q_chain(ssl)
            attn_kb(0, 0, 4 * ssl, 4 * (ssl + 1), po0)
        attn_finish(0, po0)
        # rest of wk (tiles 2,3) for the later K passes
        nc.sync.dma_start(out=wk_sb[:, :, 2:4, :], in_=wk[:, :, 2:4, :])

        attn_pair(0, 1, interleave=((8, kpass(2, 0)),))
        attn_pair(0, 2, interleave=((8, kpass(2, 1)),))
        attn_pair(0, 3, interleave=((4, kpass(2, 2)), (10, qc(4))))

        attn_pair(1, 0, interleave=((2, lambda: recip_core(0)),
                                    (4, kpass(2, 3)), (8, qc(5)),
                                    (13, mul_fc(0))))
        attn_pair(1, 1, interleave=((4, kpass(3, 0)), (8, qc(6)),
                                    (13, mul_fc(1))))
        attn_pair(1, 2, interleave=((4, kpass(3, 1)), (8, qc(7)),
                                    (13, mul_fc(2))))
        attn_pair(1, 3, interleave=((4, kpass(3, 2)), (8, qc(8)),
                                    (13, mul_fc(3))))

        attn_pair(2, 0, interleave=((2, lambda: recip_core(1)),
                                    (4, kpass(3, 3)), (8, qc(9)),
                                    (13, mul_fc(4))))
        attn_pair(2, 1, interleave=((6, qc(10)), (13, mul_fc(5))))
        attn_pair(2, 2, interleave=((6, qc(11)), (13, mul_fc(6))))
        attn_pair(2, 3, interleave=((6, qc(12)), (13, mul_fc(7))))

        attn_pair(3, 0, interleave=((2, lambda: recip_core(2)),
                                    (6, qc(13)), (13, mul_fc(8))))
        attn_pair(3, 1, interleave=((6, qc(14)), (13, mul_fc(9))))
        attn_pair(3, 2, interleave=((6, qc(15)), (13, mul_fc(10))))
        attn_pair(3, 3, interleave=((13, mul_fc(11)),))

        # wo ring buffer reuses the dead wv slot: 4 chunks of
        # [128, 2, 1024], 4-deep prefetch via subtile WAR
        wo_ring = consts.tile([128, 8, 1024], bf16, tag="wv")

        def wo_dma(k, half, fcg):
            j = 2 * (k % 4)
            nc.sync.dma_start(
                out=wo_ring[:, j:j + 2, :],
                in_=wo[:, 2 * fcg:2 * fcg + 2, 1024 * half:1024 * (half + 1)])
            return wo_ring[:, j:j + 2, :]

        wo_pre = [wo_dma(0, 0, 0), wo_dma(1, 0, 1)]

        # warm-keepers bridge the final reciprocal chain (PE HAM clock)
        for i in range(8):
            sdum = psum.tile([128, 1024], f32, tag="s", name=f"sdum{i}")
            nc.tensor.matmul(
                sdum[0:65, 0:512], lhsT=vaug_sb[:, 0, 0:65],
                rhs=qt_sb[:, 0, :], start=True, stop=True,
            )
        recip_core(3)
        for fc in (12, 13, 14, 15):
            mul_fc(fc)()

        # ---- output projection: out[tok, D] = o_norm @ wo.T
        # 8 held accumulators: psum 2 tiles -> 4 halves, opsum 2, apsum
        # [128,1024] -> 2 halves; wo streamed in [128, 2fc, 1024] chunks.
        for half in range(2):
            accs = []
            s_ts = []
            for i in range(2):
                s_t = psum.tile([128, 1024], f32, tag="s", name=f"oas{half}_{i}")
                s_ts.append(s_t)
                accs.append(s_t[:, 0:512])
                accs.append(s_t[:, 512:1024])
            o_ts = [opsum.tile([128, 512], f32, tag="o", name=f"oao{half}_{i}")
                    for i in range(2)]
            accs.extend(o_ts)
            po_t = apsum.tile([128, 1024], f32, tag="po", name=f"oap{half}")
            accs.append(po_t[:, 0:512])
            accs.append(po_t[:, 512:1024])
            # accs[tb2*2+dc2] covers out[tb2*128:+128, 1024*half+512*dc2:+512]
            def evac(tb2):
                if tb2 == 2:
                    osb = outw.tile([128, 1024], f32, tag="osb",
                                    name=f"ob{half}_2")
                    nc.vector.tensor_copy(osb[:, 0:512], o_ts[0])
                    nc.vector.tensor_copy(osb[:, 512:1024], o_ts[1])
                else:
                    src_t = {0: s_ts[0], 1: s_ts[1], 3: po_t}[tb2]
                    osb = outw.tile([128, 1024], f32, tag="osb",
                                    name=f"ob{half}_{tb2}")
                    nc.vector.tensor_copy(osb, src_t[:])
                nc.gpsimd.dma_start(
                    out=out[tb2 * 128:(tb2 + 1) * 128,
                            1024 * half:1024 * (half + 1)],
                    in_=osb)

            for fcg in range(8):
                k = half * 8 + fcg
                if k < 2:
                    wo_ch = wo_pre[k]
                else:
                    wo_ch = wo_dma(k, half, fcg)
                for fl in range(2):
                    fc = 2 * fcg + fl
                    for tb2 in range(4):
                        tsl = slice(tb2 * 128, (tb2 + 1) * 128)
                        for dc2 in range(2):
                            nc.tensor.matmul(
                                accs[tb2 * 2 + dc2],
                                lhsT=ot_sb[:, fc, tsl],
                                rhs=wo_ch[:, fl, 512 * dc2:512 * (dc2 + 1)],
                                start=(fc == 0), stop=(fc == DC - 1),
                            )
                        if half == 1 and fc == DC - 1:
                            evac(tb2)
            if half == 0:
                for tb2 in (0, 1, 2, 3):
                    evac(tb2)

    nc.compile()
    return nc


def _prep_shared(freqs_cos, freqs_sin, wqkv, wo):
    """Weight/table prep shared by all cores (token rotation applied later)."""
    cs = np.asarray(freqs_cos)[:, 0, :]  # [S, 64] (already repeat-2 layout)
    sn = np.asarray(freqs_sin)[:, 0, :]
    cos_h = np.empty((128, S), np.float32)
    sin_h = np.empty((128, S), np.float32)
    for p in range(128):
        cos_h[p] = cs[:, p % 64]
        sin_h[p] = sn[:, p % 64] * (-1.0 if p % 2 == 0 else 1.0)

    # Q rows permuted: fc = 4t+r -> [head 8t+r | head 8t+4+r]
    qrows = []
    for t in range(4):
        for r in range(4):
            for h in (8 * t + r, 8 * t + 4 + r):
                qrows.extend(range(h * HD, (h + 1) * HD))
    wq_t = np.ascontiguousarray(wqkv[qrows, :].T)  # [D, 2048]
    wq_h = np.ascontiguousarray(
        wq_t.reshape(DC, 128, DC, 128).transpose(1, 2, 0, 3)).astype(BF16)

    # K rows: tile t holds groups (2t | 2t+1)
    krows = []
    for t in range(4):
        for g in (2 * t, 2 * t + 1):
            krows.extend(range(H * HD + g * HD, H * HD + (g + 1) * HD))
    wk_t = np.ascontiguousarray(wqkv[krows, :].T)  # [D, 512]
    wk_h = np.ascontiguousarray(
        wk_t.reshape(DC, 128, 4, 128).transpose(1, 0, 2, 3)).astype(BF16)

    # V rows natural group order (cols t*128 : A 64 | B 64)
    vrows = list(range((H + G) * HD, (H + 2 * G) * HD))
    wv_t = np.ascontiguousarray(wqkv[vrows, :].T)  # [D, 512]
    wv_h = np.ascontiguousarray(
        wv_t.reshape(DC, 128, 512).transpose(1, 0, 2)).astype(BF16)

    # wo rhs: wo_h[p, fc, dcol] = wo[dcol, feat(fc, p)]
    feat = np.empty(D, np.int64)
    for fc in range(DC):
        t, r = divmod(fc, 4)
        for p in range(128):
            h = 8 * t + r + (4 if p >= 64 else 0)
            feat[fc * 128 + p] = h * HD + (p % 64)
    wo_h = np.ascontiguousarray(
        np.asarray(wo)[:, feat].T.reshape(DC, 128, D).transpose(1, 0, 2)
    ).astype(BF16)
    return cos_h, sin_h, wq_h, wk_h, wv_h, wo_h


def _prep_inputs(x, freqs_cos, freqs_sin, wqkv, wo):
    cos_h, sin_h, wq_h, wk_h, wv_h, wo_h = _prep_shared(
        freqs_cos, freqs_sin, wqkv, wo)
    x = np.asarray(x)
    ins = []
    for c in range(N_CORES):
        b, t4 = divmod(c, 4)
        q0 = t4 * TOK
        rot = (np.arange(S) + q0) % S  # own tokens land at cols 0:512
        xt_h = np.ascontiguousarray(
            x[b].T[:, rot].reshape(DC, 128, S).transpose(1, 0, 2)).astype(BF16)
        ins.append({
            "xt": xt_h,
            "wq": wq_h, "wk": wk_h, "wv": wv_h, "wo": wo_h,
            "cosr": np.ascontiguousarray(cos_h[:, rot]).astype(BF16),
            "sinr": np.ascontiguousarray(sin_h[:, rot]).astype(BF16),
        })
    return ins


TRACE = False


def kernel(x, freqs_cos, freqs_sin, wqkv, wo):
    if "nc" not in _CACHE:
        _CACHE["nc"] = _build()
    nc = _CACHE["nc"]
    ins = _prep_inputs(x, freqs_cos, freqs_sin, wqkv, wo)
    res = run_bass_kernel_spmd(nc, ins, list(range(N_CORES)), trace=TRACE)
    _CACHE["res"] = res
    out = np.empty((B, S, D), np.float32)
    for c in range(N_CORES):
        b, t4 = divmod(c, 4)
        out[b, t4 * TOK:(t4 + 1) * TOK, :] = res.results[c]["out"]
    return out


if __name__ == "__main__":
    rng = np.random.default_rng(0)
    x = rng.normal(size=(B, S, D)).astype(np.float32)
    fc_ = rng.random(size=(S, 1, HD)).astype(np.float32)
    fs_ = rng.random(size=(S, 1, HD)).astype(np.float32)
    wq_ = rng.normal(size=(3072, D)).astype(np.float32) * 0.02
    wo_ = rng.normal(size=(D, D)).astype(np.float32) * 0.02
    o = kernel(x, fc_, fs_, wq_, wo_)
    print(o.shape, o.dtype)


# revision 22
# speedup vs baseline: 1.0024x; 1.0024x over previous
"""GQA attention (B=2, S=2048, D=2048, H=32, G=8, hd=64) on 8 TRN2 cores.

ZERO-COLLECTIVE sharding: core c owns (batch b=c//4, token block
q0=512*(c%4)). Each core computes the FULL output slice out[b,
q0:q0+512, :] independently: full K/V over all S (replicated within a
batch group), Q only for its own 512 tokens, attention + output
projection fully local. Collectives are deliberately avoided: on this
runtime an AllGather trips a GPIO power throttle that caps the PE clock
at 81% for most of the kernel (~+100us) on top of ~60-100us latency.

Per-core SPMD uniformity: the token axis of x^T (and the RoPE tables) is
ROTATED by q0 on the host, so "own tokens" are always columns 0:512.
Attention is permutation-invariant over keys; RoPE phases ride with the
rotation.

Schedule: x^T streams in 512-token chunks; attention round 0 pair 0
rides the first pass chunk-by-chunk (K tiles 0,1 + V + Q per chunk), so
the exp stream starts at ~40us instead of waiting for the full 8.4 MB
x^T load. K tiles 2,3 and remaining Q chains interleave into later
pairs' PE slack, keeping the PE dense (HAM clock stays 8/8). Softmax
denominators accumulate free in PSUM row 64 via an augmented-V ones
column; reciprocals run on DVE (reciprocal_approx_fast) so ACT does
pure exp with a single table load (preheated).

PSUM: scores 2x[128,1024] (4 banks) + chain accumulators 2x[128,512]
(2) + PV accumulator [65,1024] (2, heads A|B side by side). The output
projection reuses all 8 banks as 8 held accumulators while wo streams
through SBUF in 0.5 MB chunks.
"""

import sys

sys.path.insert(0, "/opt/trn_rl_repo")

import numpy as np
import ml_dtypes

import concourse.bass as bass
import concourse.tile as tile
from concourse import bacc, mybir
from concourse.bass_utils import run_bass_kernel_spmd

BF16 = ml_dtypes.bfloat16
B, S, D = 2, 2048, 2048
H, G, HD = 32, 8, 64
DC = D // 128  # 16 dim chunks
N_CORES = 8
TOK = 512  # own tokens per core

_CACHE = {}


def _build():
    f32 = mybir.dt.float32
    bf16 = mybir.dt.bfloat16
    nc = bacc.Bacc("TRN2", target_bir_lowering=False, debug=False, num_devices=N_CORES)

    xt = nc.dram_tensor("xt", [128, DC, S], bf16, kind="ExternalInput").ap()
    wq = nc.dram_tensor("wq", [128, DC, DC, 128], bf16, kind="ExternalInput").ap()
    wk = nc.dram_tensor("wk", [128, DC, 4, 128], bf16, kind="ExternalInput").ap()
    wv = nc.dram_tensor("wv", [128, DC, 512], bf16, kind="ExternalInput").ap()
    cosr = nc.dram_tensor("cosr", [128, S], bf16, kind="ExternalInput").ap()
    sinr = nc.dram_tensor("sinr", [128, S], bf16, kind="ExternalInput").ap()
    wo = nc.dram_tensor("wo", [128, DC, D], bf16, kind="ExternalInput").ap()
    out = nc.dram_tensor("out", [TOK, D], f32, kind="ExternalOutput").ap()

    Exp = mybir.ActivationFunctionType.Exp
    swap_mask = [i ^ 1 for i in range(32)]
    scale = float(1.0 / np.sqrt(HD))

    from contextlib import ExitStack
    with tile.TileContext(nc) as tc, ExitStack() as ctx:
        consts = ctx.enter_context(tc.tile_pool(name="consts", bufs=1))
        xtfp = ctx.enter_context(tc.tile_pool(name="xtfp", bufs=2))
        wqp = ctx.enter_context(tc.tile_pool(name="wqp", bufs=2))
        io = ctx.enter_context(tc.tile_pool(name="io", bufs=2))
        work = ctx.enter_context(tc.tile_pool(name="work", bufs=3))
        outw = ctx.enter_context(tc.tile_pool(name="outw", bufs=2))
        psum = ctx.enter_context(tc.tile_pool(name="psum", bufs=2, space="PSUM"))
        opsum = ctx.enter_context(tc.tile_pool(name="opsum", bufs=2, space="PSUM"))
        apsum = ctx.enter_context(tc.tile_pool(name="apsum", bufs=1, space="PSUM"))
        dram = ctx.enter_context(tc.tile_pool(name="dram", bufs=1, space="DRAM"))

        # ---- header loads, ordered to unlock K-chunk0 -> Q0 -> V-chunk0.
        # xf0 (own tokens) is persistent: all 16 Q chains read it.
        wk_sb = consts.tile([128, DC, 4, 128], bf16, tag="wk")
        nc.sync.dma_start(out=wk_sb[:, :, 0:1, :], in_=wk[:, :, 0:1, :])
        xf0 = consts.tile([128, DC, 512], bf16, tag="xf0")
        nc.sync.dma_start(out=xf0[:], in_=xt[:, :, 0:512])
        nc.sync.dma_start(out=wk_sb[:, :, 1:2, :], in_=wk[:, :, 1:2, :])
        cosf_sb = consts.tile([128, S], bf16, tag="cosf")
        nc.sync.dma_start(out=cosf_sb[:], in_=cosr[:])
        sinf_sb = consts.tile([128, S], bf16, tag="sinf")
        nc.sync.dma_start(out=sinf_sb[:], in_=sinr[:])
        # wq0 is emitted by q_chain(0) below -> lands here in queue order
        wv_sb = consts.tile([128, DC, 512], bf16, tag="wv")

        # preheat the exp table set (one ACT_TABLE_LOAD, early)
        preheat = io.tile([1, 64], bf16, tag="pre")
        nc.vector.memset(preheat[:], 0.0)
        nc.scalar.activation(preheat[:], preheat[:], Exp)

        kt_sb = consts.tile([128, 4, S], bf16, tag="kt")
        vaug_sb = consts.tile([128, DC, 520], bf16, tag="vaug")
        qt_sb = consts.tile([128, DC, TOK], bf16, tag="qt")
        ot_sb = consts.tile([128, DC, TOK], bf16, tag="ot")
        dstage = consts.tile([97, 4096], f32, tag="dstage")
        nc.vector.memset(dstage[:], 1.0)
        ddram = dram.tile([4, 4096], f32, tag="dd", name="dd")
        # ones columns of augmented V (denominator accumulators)
        for col in (64, 129, 194, 259, 324, 389, 454, 519):
            nc.vector.memset(vaug_sb[:, :, col:col + 1], 1.0)

        def rope(ap, cs, sn):
            sw = io.tile([128, 512], bf16, tag="rsw")
            nc.vector.stream_shuffle(sw, ap, swap_mask)
            nc.vector.tensor_mul(sw, sw, sn)
            tmp = io.tile([128, 512], bf16, tag="rtmp")
            nc.vector.tensor_mul(tmp, ap, cs)
            nc.vector.tensor_add(ap, sw, tmp)

        def xf_dma(ssl, name):
            xf = xtfp.tile([128, DC, 512], bf16, tag="xf", name=name)
            nc.sync.dma_start(out=xf[:], in_=xt[:, :, 512 * ssl:512 * (ssl + 1)])
            return xf

        def pass_K(t, ssl, xf):
            sl = slice(512 * ssl, 512 * (ssl + 1))
            ps = opsum.tile([128, 512], f32, tag="o", name=f"psk{t}_{ssl}")
            for c in range(DC):
                nc.tensor.matmul(
                    ps, lhsT=wk_sb[:, c, t, :], rhs=xf[:, c, :],
                    start=(c == 0), stop=(c == DC - 1),
                )
            nc.vector.tensor_copy(kt_sb[:, t, sl], ps)
            rope(kt_sb[:, t, sl], cosf_sb[:, sl], sinf_sb[:, sl])

        def pass_V(ssl, xf):
            # full V projection for 512 tokens -> vaug cols 0:520
            for tbl in range(4):
                tb = 4 * ssl + tbl
                ps = opsum.tile([128, 512], f32, tag="o", name=f"psv{tb}")
                for c in range(DC):
                    nc.tensor.matmul(
                        ps,
                        lhsT=xf[:, c, tbl * 128:(tbl + 1) * 128],
                        rhs=wv_sb[:, c, :],
                        start=(c == 0), stop=(c == DC - 1),
                    )
                for tt in range(4):
                    base = 130 * tt
                    nc.vector.tensor_copy(
                        vaug_sb[:, tb, base:base + 64],
                        ps[:, 128 * tt:128 * tt + 64])
                    nc.vector.tensor_copy(
                        vaug_sb[:, tb, base + 65:base + 129],
                        ps[:, 128 * tt + 64:128 * tt + 128])

        def q_chain(fc):
            wq_t = wqp.tile([128, DC, 128], bf16, tag="wq", name=f"wqt{fc}")
            nc.sync.dma_start(out=wq_t[:], in_=wq[:, fc, :, :])
            ps = opsum.tile([128, 512], f32, tag="o", name=f"psq{fc}")
            for c in range(DC):
                nc.tensor.matmul(
                    ps, lhsT=wq_t[:, c, :], rhs=xf0[:, c, :],
                    start=(c == 0), stop=(c == DC - 1),
                )
            nc.vector.tensor_copy(qt_sb[:, fc, :], ps)
            rope(qt_sb[:, fc, :], cosf_sb[:, 0:512], sinf_sb[:, 0:512])

        def attn_kb(t, fc, kb0, kb1, po):
            for kb in range(kb0, kb1):
                ksl = slice(kb * 128, (kb + 1) * 128)
                s = psum.tile([128, 1024], f32, tag="s", name=f"s{fc}_{kb}")
                nc.tensor.matmul(
                    s[:, 0:512], lhsT=kt_sb[0:64, t, ksl],
                    rhs=qt_sb[0:64, fc, :],
                    start=True, stop=True, tile_position=(0, 0),
                )
                nc.tensor.matmul(
                    s[:, 512:1024], lhsT=kt_sb[64:128, t, ksl],
                    rhs=qt_sb[64:128, fc, :],
                    start=True, stop=True, tile_position=(64, 0),
                )
                p = work.tile([128, 1024], bf16, tag="p", name=f"p{fc}_{kb}")
                nc.scalar.activation(p, s, Exp, scale=scale)
                nc.tensor.matmul(
                    po[:, 0:512], lhsT=vaug_sb[:, kb, 130 * t:130 * t + 65],
                    rhs=p[:, 0:512],
                    start=(kb == 0), stop=(kb == DC - 1),
                )
                nc.tensor.matmul(
                    po[:, 512:1024], lhsT=vaug_sb[:, kb, 130 * t + 65:130 * t + 130],
                    rhs=p[:, 512:1024],
                    start=(kb == 0), stop=(kb == DC - 1),
                )

        def attn_finish(fc, po):
            nc.vector.tensor_copy(ot_sb[0:64, fc, :], po[0:64, 0:512])
            nc.vector.tensor_copy(ot_sb[64:128, fc, :], po[0:64, 512:1024])
            dp = 32 * (fc % 4)
            df = (fc // 4) * 1024
            nc.vector.tensor_copy(dstage[dp:dp + 1, df:df + 1024],
                                  po[64:65, 0:1024])

        def attn_pair(t, r, interleave=()):
            fc = 4 * t + r
            po = apsum.tile([65, 1024], f32, tag="po", name=f"po{fc}")
            points = sorted(set(k for k, _ in interleave))
            cuts = [0] + points + [DC]
            for i in range(len(cuts) - 1):
                if i > 0:
                    for k, fn in interleave:
                        if k == cuts[i]:
                            fn()
                attn_kb(t, fc, cuts[i], cuts[i + 1], po)
            attn_finish(fc, po)

        def recip_core(t):
            hs = slice(1024 * t, 1024 * (t + 1))
            nc.vector.reciprocal_approx_fast(out=dstage[:, hs], in_=dstage[:, hs])
            for rr in range(4):
                nc.sync.dma_start(out=ddram[rr:rr + 1, hs],
                                  in_=dstage[32 * rr:32 * rr + 1, hs])

        def mul_fc(fc):
            # normalize ot chunk fc by 1/d (broadcast via DRAM bounce rows);
            # spread across later pairs so DVE never spikes at round edges
            def fn():
                dp = fc % 4
                df = (fc // 4) * 1024
                r2 = io.tile([128, TOK], bf16, tag="r2", name=f"r2_{fc}")
                nc.gpsimd.dma_start(
                    out=r2[0:64, :],
                    in_=ddram[dp:dp + 1, df:df + 512].partition_broadcast(64))
                nc.gpsimd.dma_start(
                    out=r2[64:128, :],
                    in_=ddram[dp:dp + 1, df + 512:df + 1024].partition_broadcast(64))
                nc.vector.tensor_mul(ot_sb[:, fc, :], ot_sb[:, fc, :], r2)
            return fn

        def kpass(t, ssl):
            def fn():
                xf = xf_dma(ssl, f"xfk{t}_{ssl}")
                pass_K(t, ssl, xf)
            return fn

        def qc(fc):
            return lambda: q_chain(fc)

        # ================= schedule =================
        # round 0 pair 0 rides the first streaming pass chunk by chunk:
        # per 512-token chunk: K tiles 0,1 + V + one Q chain + 4 kb of
        # attention. Own tokens are chunk 0 (rotated layout).
        po0 = apsum.tile([65, 1024], f32, tag="po", name="po0")

        pass_K(0, 0, xf0)
        q_chain(0)
        nc.sync.dma_start(out=wv_sb[:], in_=wv[:])  # queued after wq0
        pass_K(1, 0, xf0)
        pass_V(0, xf0)
        attn_kb(0, 0, 0, 4, po0)

        for ssl in (1, 2, 3):
            xf = xf_dma(ssl, f"xfa{ssl}")
            pass_K(0, ssl, xf)
            pass_K(1, ssl, xf)
            pass_V(ssl, xf)
            q_chain(ssl)
            attn_kb(0, 0, 4 * ssl, 4 * (ssl + 1), po0)
        attn_finish(0, po0)
        # rest of wk (tiles 2,3) for the later K passes
        nc.sync.dma_start(out=wk_sb[:, :, 2:4, :], in_=wk[:, :, 2:4, :])

        attn_pair(0, 1, interleave=((8, kpass(2, 0)),))
        attn_pair(0, 2, interleave=((8, kpass(2, 1)),))
        attn_pair(0, 3, interleave=((4, kpass(2, 2)), (10, qc(4))))

        attn_pair(1, 0, interleave=((2, lambda: recip_core(0)),
                                    (4, kpass(2, 3)), (8, qc(5)),
                                    (13, mul_fc(0))))
        attn_pair(1, 1, interleave=((4, kpass(3, 0)), (8, qc(6)),
                                    (13, mul_fc(1))))
        attn_pair(1, 2, interleave=((4, kpass(3, 1)), (8, qc(7)),
                                    (13, mul_fc(2))))
        attn_pair(1, 3, interleave=((4, kpass(3, 2)), (8, qc(8)),
                                    (13, mul_fc(3))))

        attn_pair(2, 0, interleave=((2, lambda: recip_core(1)),
                                    (4, kpass(3, 3)), (8, qc(9)),
                                    (13, mul_fc(4))))
        attn_pair(2, 1, interleave=((6, qc(10)), (13, mul_fc(5))))
        attn_pair(2, 2, interleave=((6, qc(11)), (13, mul_fc(6))))
        attn_pair(2, 3, interleave=((6, qc(12)), (13, mul_fc(7))))

        attn_pair(3, 0, interleave=((2, lambda: recip_core(2)),
                                    (6, qc(13)), (13, mul_fc(8))))
        attn_pair(3, 1, interleave=((6, qc(14)), (13, mul_fc(9))))
        attn_pair(3, 2, interleave=((6, qc(15)), (13, mul_fc(10))))
        attn_pair(3, 3, interleave=((13, mul_fc(11)),))

        # wo ring buffer reuses the dead wv slot: 4 chunks of
        # [128, 2, 1024], 4-deep prefetch via subtile WAR
        wo_ring = consts.tile([128, 8, 1024], bf16, tag="wv")

        def wo_dma(k, half, fcg):
            j = 2 * (k % 4)
            nc.sync.dma_start(
                out=wo_ring[:, j:j + 2, :],
                in_=wo[:, 2 * fcg:2 * fcg + 2, 1024 * half:1024 * (half + 1)])
            return wo_ring[:, j:j + 2, :]

        wo_pre = [wo_dma(0, 0, 0), wo_dma(1, 0, 1)]

        # warm-keepers bridge the final reciprocal chain (PE HAM clock)
        for i in range(8):
            sdum = psum.tile([128, 1024], f32, tag="s", name=f"sdum{i}")
            nc.tensor.matmul(
                sdum[0:65, 0:512], lhsT=vaug_sb[:, 0, 0:65],
                rhs=qt_sb[:, 0, :], start=True, stop=True,
            )
        recip_core(3)
        for fc in (12, 13, 14, 15):
            mul_fc(fc)()

        # ---- output projection: out[tok, D] = o_norm @ wo.T
        # 8 held accumulators: psum 2 tiles -> 4 halves, opsum 2, apsum
        # [128,1024] -> 2 halves; wo streamed in [128, 2fc, 1024] chunks.
        for half in range(2):
            accs = []
            s_ts = []
            for i in range(2):
                s_t = psum.tile([128, 1024], f32, tag="s", name=f"oas{half}_{i}")
                s_ts.append(s_t)
                accs.append(s_t[:, 0:512])
                accs.append(s_t[:, 512:1024])
            o_ts = [opsum.tile([128, 512], f32, tag="o", name=f"oao{half}_{i}")
                    for i in range(2)]
            accs.extend(o_ts)
            po_t = apsum.tile([128, 1024], f32, tag="po", name=f"oap{half}")
            accs.append(po_t[:, 0:512])
            accs.append(po_t[:, 512:1024])
            # accs[tb2*2+dc2] covers out[tb2*128:+128, 1024*half+512*dc2:+512]
            for fcg in range(8):
                k = half * 8 + fcg
                if k < 2:
                    wo_ch = wo_pre[k]
                else:
                    wo_ch = wo_dma(k, half, fcg)
                for fl in range(2):
                    fc = 2 * fcg + fl
                    for tb2 in range(4):
                        tsl = slice(tb2 * 128, (tb2 + 1) * 128)
                        for dc2 in range(2):
                            nc.tensor.matmul(
                                accs[tb2 * 2 + dc2],
                                lhsT=ot_sb[:, fc, tsl],
                                rhs=wo_ch[:, fl, 512 * dc2:512 * (dc2 + 1)],
                                start=(fc == 0), stop=(fc == DC - 1),
                            )
            # evacuate: merged copies + 1024-wide DMAs (gpsimd queue)
            for tb2, src_t in ((0, s_ts[0][:]), (1, s_ts[1][:]), (3, po_t[:])):
                osb = outw.tile([128, 1024], f32, tag="osb",
                                name=f"ob{half}_{tb2}")
                nc.vector.tensor_copy(osb, src_t)
                nc.gpsimd.dma_start(
                    out=out[tb2 * 128:(tb2 + 1) * 128,
                            1024 * half:1024 * (half + 1)],
                    in_=osb)
            osb = outw.tile([128, 1024], f32, tag="osb", name=f"ob{half}_2")
            nc.vector.tensor_copy(osb[:, 0:512], o_ts[0])
            nc.vector.tensor_copy(osb[:, 512:1024], o_ts[1])
            nc.gpsimd.dma_start(
                out=out[256:384, 1024 * half:1024 * (half + 1)], in_=osb)

    nc.compile()
    return nc


def _prep_shared(freqs_cos, freqs_sin, wqkv, wo):
    """Weight/table prep shared by all cores (token rotation applied later)."""
    cs = np.asarray(freqs_cos)[:, 0, :]  # [S, 64] (already repeat-2 layout)
    sn = np.asarray(freqs_sin)[:, 0, :]
    cos_h = np.empty((128, S), np.float32)
    sin_h = np.empty((128, S), np.float32)
    for p in range(128):
        cos_h[p] = cs[:, p % 64]
        sin_h[p] = sn[:, p % 64] * (-1.0 if p % 2 == 0 else 1.0)

    # Q rows permuted: fc = 4t+r -> [head 8t+r | head 8t+4+r]
    qrows = []
    for t in range(4):
        for r in range(4):
            for h in (8 * t + r, 8 * t + 4 + r):
                qrows.extend(range(h * HD, (h + 1) * HD))
    wq_t = np.ascontiguousarray(wqkv[qrows, :].T)  # [D, 2048]
    wq_h = np.ascontiguousarray(
        wq_t.reshape(DC, 128, DC, 128).transpose(1, 2, 0, 3)).astype(BF16)

    # K rows: tile t holds groups (2t | 2t+1)
    krows = []
    for t in range(4):
        for g in (2 * t, 2 * t + 1):
            krows.extend(range(H * HD + g * HD, H * HD + (g + 1) * HD))
    wk_t = np.ascontiguousarray(wqkv[krows, :].T)  # [D, 512]
    wk_h = np.ascontiguousarray(
        wk_t.reshape(DC, 128, 4, 128).transpose(1, 0, 2, 3)).astype(BF16)

    # V rows natural group order (cols t*128 : A 64 | B 64)
    vrows = list(range((H + G) * HD, (H + 2 * G) * HD))
    wv_t = np.ascontiguousarray(wqkv[vrows, :].T)  # [D, 512]
    wv_h = np.ascontiguousarray(
        wv_t.reshape(DC, 128, 512).transpose(1, 0, 2)).astype(BF16)

    # wo rhs: wo_h[p, fc, dcol] = wo[dcol, feat(fc, p)]
    feat = np.empty(D, np.int64)
    for fc in range(DC):
        t, r = divmod(fc, 4)
        for p in range(128):
            h = 8 * t + r + (4 if p >= 64 else 0)
            feat[fc * 128 + p] = h * HD + (p % 64)
    wo_h = np.ascontiguousarray(
        np.asarray(wo)[:, feat].T.reshape(DC, 128, D).transpose(1, 0, 2)
    ).astype(BF16)
    return cos_h, sin_h, wq_h, wk_h, wv_h, wo_h


def _prep_inputs(x, freqs_cos, freqs_sin, wqkv, wo):
    cos_h, sin_h, wq_h, wk_h, wv_h, wo_h = _prep_shared(
        freqs_cos, freqs_sin, wqkv, wo)
    x = np.asarray(x)
    ins = []
    for c in range(N_CORES):
        b, t4 = divmod(c, 4)
        q0 = t4 * TOK
        rot = (np.arange(S) + q0) % S  # own tokens land at cols 0:512
        xt_h = np.ascontiguousarray(
            x[b].T[:, rot].reshape(DC, 128, S).transpose(1, 0, 2)).astype(BF16)
        ins.append({
            "xt": xt_h,
            "wq": wq_h, "wk": wk_h, "wv": wv_h, "wo": wo_h,
            "cosr": np.ascontiguousarray(cos_h[:, rot]).astype(BF16),
            "sinr": np.ascontiguousarray(sin_h[:, rot]).astype(BF16),
        })
    return ins


TRACE = False


def kernel(x, freqs_cos, freqs_sin, wqkv, wo):
    if "nc" not in _CACHE:
        _CACHE["nc"] = _build()
    nc = _CACHE["nc"]
    ins = _prep_inputs(x, freqs_cos, freqs_sin, wqkv, wo)
    res = run_bass_kernel_spmd(nc, ins, list(range(N_CORES)), trace=TRACE)
    _CACHE["res"] = res
    out = np.empty((B, S, D), np.float32)
    for c in range(N_CORES):
        b, t4 = divmod(c, 4)
        out[b, t4 * TOK:(t4 + 1) * TOK, :] = res.results[c]["out"]
    return out


if __name__ == "__main__":
    rng = np.random.default_rng(0)
    x = rng.normal(size=(B, S, D)).astype(np.float32)
    fc_ = rng.random(size=(S, 1, HD)).astype(np.float32)
    fs_ = rng.random(size=(S, 1, HD)).astype(np.float32)
    wq_ = rng.normal(size=(3072, D)).astype(np.float32) * 0.02
    wo_ = rng.normal(size=(D, D)).astype(np.float32) * 0.02
    o = kernel(x, fc_, fs_, wq_, wo_)
    print(o.shape, o.dtype)


# revision 24
# speedup vs baseline: 1.0091x; 1.0067x over previous
"""GQA attention (B=2, S=2048, D=2048, H=32, G=8, hd=64) on 8 TRN2 cores.

ZERO-COLLECTIVE sharding: core c owns (batch b=c//4, token block
q0=512*(c%4)). Each core computes the FULL output slice out[b,
q0:q0+512, :] independently: full K/V over all S (replicated within a
batch group), Q only for its own 512 tokens, attention + output
projection fully local. Collectives are deliberately avoided: on this
runtime an AllGather trips a GPIO power throttle that caps the PE clock
at 81% for most of the kernel (~+100us) on top of ~60-100us latency.

Per-core SPMD uniformity: the token axis of x^T (and the RoPE tables) is
ROTATED by q0 on the host, so "own tokens" are always columns 0:512.
Attention is permutation-invariant over keys; RoPE phases ride with the
rotation.

Schedule: x^T streams in 512-token chunks; attention round 0 pair 0
rides the first pass chunk-by-chunk (K tiles 0,1 + V + Q per chunk), so
the exp stream starts at ~40us instead of waiting for the full 8.4 MB
x^T load. K tiles 2,3 and remaining Q chains interleave into later
pairs' PE slack, keeping the PE dense (HAM clock stays 8/8). Softmax
denominators accumulate free in PSUM row 64 via an augmented-V ones
column; reciprocals run on DVE (reciprocal_approx_fast) so ACT does
pure exp with a single table load (preheated).

PSUM: scores 2x[128,1024] (4 banks) + chain accumulators 2x[128,512]
(2) + PV accumulator [65,1024] (2, heads A|B side by side). The output
projection reuses all 8 banks as 8 held accumulators while wo streams
through SBUF in 0.5 MB chunks.
"""

import sys

sys.path.insert(0, "/opt/trn_rl_repo")

import numpy as np
import ml_dtypes

import concourse.bass as bass
import concourse.tile as tile
from concourse import bacc, mybir
from concourse.bass_utils import run_bass_kernel_spmd

BF16 = ml_dtypes.bfloat16
B, S, D = 2, 2048, 2048
H, G, HD = 32, 8, 64
DC = D // 128  # 16 dim chunks
N_CORES = 8
TOK = 512  # own tokens per core

_CACHE = {}


def _build():
    f32 = mybir.dt.float32
    bf16 = mybir.dt.bfloat16
    nc = bacc.Bacc("TRN2", target_bir_lowering=False, debug=False, num_devices=N_CORES)

    xt = nc.dram_tensor("xt", [128, DC, S], bf16, kind="ExternalInput").ap()
    wq = nc.dram_tensor("wq", [128, DC, DC, 128], bf16, kind="ExternalInput").ap()
    wk = nc.dram_tensor("wk", [128, DC, 4, 128], bf16, kind="ExternalInput").ap()
    wv = nc.dram_tensor("wv", [128, DC, 512], bf16, kind="ExternalInput").ap()
    cosr = nc.dram_tensor("cosr", [128, S], bf16, kind="ExternalInput").ap()
    sinr = nc.dram_tensor("sinr", [128, S], bf16, kind="ExternalInput").ap()
    wo = nc.dram_tensor("wo", [128, DC, D], bf16, kind="ExternalInput").ap()
    out = nc.dram_tensor("out", [TOK, D], f32, kind="ExternalOutput").ap()

    Exp = mybir.ActivationFunctionType.Exp
    swap_mask = [i ^ 1 for i in range(32)]
    scale = float(1.0 / np.sqrt(HD))

    from contextlib import ExitStack
    with tile.TileContext(nc) as tc, ExitStack() as ctx:
        consts = ctx.enter_context(tc.tile_pool(name="consts", bufs=1))
        xtfp = ctx.enter_context(tc.tile_pool(name="xtfp", bufs=2))
        wqp = ctx.enter_context(tc.tile_pool(name="wqp", bufs=2))
        io = ctx.enter_context(tc.tile_pool(name="io", bufs=2))
        work = ctx.enter_context(tc.tile_pool(name="work", bufs=3))
        outw = ctx.enter_context(tc.tile_pool(name="outw", bufs=2))
        psum = ctx.enter_context(tc.tile_pool(name="psum", bufs=2, space="PSUM"))
        opsum = ctx.enter_context(tc.tile_pool(name="opsum", bufs=2, space="PSUM"))
        apsum = ctx.enter_context(tc.tile_pool(name="apsum", bufs=1, space="PSUM"))
        dram = ctx.enter_context(tc.tile_pool(name="dram", bufs=1, space="DRAM"))

        # ---- header loads, ordered to unlock K-chunk0 -> Q0 -> V-chunk0.
        # xf0 (own tokens) is persistent: all 16 Q chains read it.
        # header loads fan across engine DMA queues so they land in
        # parallel: big xf0 on sync, wk on vector, tables+wv on scalar
        wk_sb = consts.tile([128, DC, 4, 128], bf16, tag="wk")
        nc.scalar.dma_start(out=wk_sb[:, :, 0:2, :], in_=wk[:, :, 0:2, :])
        xf0 = consts.tile([128, DC, 512], bf16, tag="xf0")
        nc.sync.dma_start(out=xf0[:], in_=xt[:, :, 0:512])
        cosf_sb = consts.tile([128, S], bf16, tag="cosf")
        nc.scalar.dma_start(out=cosf_sb[:], in_=cosr[:])
        sinf_sb = consts.tile([128, S], bf16, tag="sinf")
        nc.scalar.dma_start(out=sinf_sb[:], in_=sinr[:])
        # wq0 is emitted by q_chain(0) below -> lands here in queue order
        wv_sb = consts.tile([128, DC, 512], bf16, tag="wv")
        nc.scalar.dma_start(out=wv_sb[:], in_=wv[:])

        # preheat the exp table set (one ACT_TABLE_LOAD, early)
        preheat = io.tile([1, 64], bf16, tag="pre")
        nc.vector.memset(preheat[:], 0.0)
        nc.scalar.activation(preheat[:], preheat[:], Exp)

        kt_sb = consts.tile([128, 4, S], bf16, tag="kt")
        vaug_sb = consts.tile([128, DC, 520], bf16, tag="vaug")
        qt_sb = consts.tile([128, DC, TOK], bf16, tag="qt")
        ot_sb = consts.tile([128, DC, TOK], bf16, tag="ot")
        dstage = consts.tile([97, 4096], f32, tag="dstage")
        nc.vector.memset(dstage[:], 1.0)
        ddram = dram.tile([4, 4096], f32, tag="dd", name="dd")
        # ones columns of augmented V (denominator accumulators)
        for col in (64, 129, 194, 259, 324, 389, 454, 519):
            nc.vector.memset(vaug_sb[:, :, col:col + 1], 1.0)

        def rope(ap, cs, sn):
            sw = io.tile([128, 512], bf16, tag="rsw")
            nc.vector.stream_shuffle(sw, ap, swap_mask)
            nc.vector.tensor_mul(sw, sw, sn)
            tmp = io.tile([128, 512], bf16, tag="rtmp")
            nc.vector.tensor_mul(tmp, ap, cs)
            nc.vector.tensor_add(ap, sw, tmp)

        def xf_dma(ssl, name):
            xf = xtfp.tile([128, DC, 512], bf16, tag="xf", name=name)
            nc.sync.dma_start(out=xf[:], in_=xt[:, :, 512 * ssl:512 * (ssl + 1)])
            return xf

        def pass_K(t, ssl, xf):
            sl = slice(512 * ssl, 512 * (ssl + 1))
            ps = opsum.tile([128, 512], f32, tag="o", name=f"psk{t}_{ssl}")
            for c in range(DC):
                nc.tensor.matmul(
                    ps, lhsT=wk_sb[:, c, t, :], rhs=xf[:, c, :],
                    start=(c == 0), stop=(c == DC - 1),
                )
            nc.vector.tensor_copy(kt_sb[:, t, sl], ps)
            rope(kt_sb[:, t, sl], cosf_sb[:, sl], sinf_sb[:, sl])

        def pass_V(ssl, xf):
            # full V projection for 512 tokens -> vaug cols 0:520
            for tbl in range(4):
                tb = 4 * ssl + tbl
                ps = opsum.tile([128, 512], f32, tag="o", name=f"psv{tb}")
                for c in range(DC):
                    nc.tensor.matmul(
                        ps,
                        lhsT=xf[:, c, tbl * 128:(tbl + 1) * 128],
                        rhs=wv_sb[:, c, :],
                        start=(c == 0), stop=(c == DC - 1),
                    )
                for tt in range(4):
                    base = 130 * tt
                    nc.vector.tensor_copy(
                        vaug_sb[:, tb, base:base + 64],
                        ps[:, 128 * tt:128 * tt + 64])
                    nc.vector.tensor_copy(
                        vaug_sb[:, tb, base + 65:base + 129],
                        ps[:, 128 * tt + 64:128 * tt + 128])

        def q_chain(fc):
            wq_t = wqp.tile([128, DC, 128], bf16, tag="wq", name=f"wqt{fc}")
            nc.sync.dma_start(out=wq_t[:], in_=wq[:, fc, :, :])
            ps = opsum.tile([128, 512], f32, tag="o", name=f"psq{fc}")
            for c in range(DC):
                nc.tensor.matmul(
                    ps, lhsT=wq_t[:, c, :], rhs=xf0[:, c, :],
                    start=(c == 0), stop=(c == DC - 1),
                )
            nc.vector.tensor_copy(qt_sb[:, fc, :], ps)
            rope(qt_sb[:, fc, :], cosf_sb[:, 0:512], sinf_sb[:, 0:512])

        def attn_kb(t, fc, kb0, kb1, po):
            for kb in range(kb0, kb1):
                ksl = slice(kb * 128, (kb + 1) * 128)
                s = psum.tile([128, 1024], f32, tag="s", name=f"s{fc}_{kb}")
                nc.tensor.matmul(
                    s[:, 0:512], lhsT=kt_sb[0:64, t, ksl],
                    rhs=qt_sb[0:64, fc, :],
                    start=True, stop=True, tile_position=(0, 0),
                )
                nc.tensor.matmul(
                    s[:, 512:1024], lhsT=kt_sb[64:128, t, ksl],
                    rhs=qt_sb[64:128, fc, :],
                    start=True, stop=True, tile_position=(64, 0),
                )
                p = work.tile([128, 1024], bf16, tag="p", name=f"p{fc}_{kb}")
                nc.scalar.activation(p, s, Exp, scale=scale)
                nc.tensor.matmul(
                    po[:, 0:512], lhsT=vaug_sb[:, kb, 130 * t:130 * t + 65],
                    rhs=p[:, 0:512],
                    start=(kb == 0), stop=(kb == DC - 1),
                )
                nc.tensor.matmul(
                    po[:, 512:1024], lhsT=vaug_sb[:, kb, 130 * t + 65:130 * t + 130],
                    rhs=p[:, 512:1024],
                    start=(kb == 0), stop=(kb == DC - 1),
                )

        def attn_finish(fc, po):
            nc.vector.tensor_copy(ot_sb[0:64, fc, :], po[0:64, 0:512])
            nc.vector.tensor_copy(ot_sb[64:128, fc, :], po[0:64, 512:1024])
            dp = 32 * (fc % 4)
            df = (fc // 4) * 1024
            nc.vector.tensor_copy(dstage[dp:dp + 1, df:df + 1024],
                                  po[64:65, 0:1024])

        def attn_pair(t, r, interleave=()):
            fc = 4 * t + r
            po = apsum.tile([65, 1024], f32, tag="po", name=f"po{fc}")
            points = sorted(set(k for k, _ in interleave))
            cuts = [0] + points + [DC]
            for i in range(len(cuts) - 1):
                if i > 0:
                    for k, fn in interleave:
                        if k == cuts[i]:
                            fn()
                attn_kb(t, fc, cuts[i], cuts[i + 1], po)
            attn_finish(fc, po)

        def recip_core(t):
            hs = slice(1024 * t, 1024 * (t + 1))
            nc.vector.reciprocal_approx_fast(out=dstage[:, hs], in_=dstage[:, hs])
            for rr in range(4):
                nc.sync.dma_start(out=ddram[rr:rr + 1, hs],
                                  in_=dstage[32 * rr:32 * rr + 1, hs])

        def mul_fc(fc):
            # normalize ot chunk fc by 1/d (broadcast via DRAM bounce rows);
            # spread across later pairs so DVE never spikes at round edges
            def fn():
                dp = fc % 4
                df = (fc // 4) * 1024
                r2 = io.tile([128, TOK], bf16, tag="r2", name=f"r2_{fc}")
                nc.gpsimd.dma_start(
                    out=r2[0:64, :],
                    in_=ddram[dp:dp + 1, df:df + 512].partition_broadcast(64))
                nc.gpsimd.dma_start(
                    out=r2[64:128, :],
                    in_=ddram[dp:dp + 1, df + 512:df + 1024].partition_broadcast(64))
                nc.vector.tensor_mul(ot_sb[:, fc, :], ot_sb[:, fc, :], r2)
            return fn

        def kpass(t, ssl):
            def fn():
                xf = xf_dma(ssl, f"xfk{t}_{ssl}")
                pass_K(t, ssl, xf)
            return fn

        def qc(fc):
            return lambda: q_chain(fc)

        # ================= schedule =================
        # round 0 pair 0 rides the first streaming pass chunk by chunk:
        # per 512-token chunk: K tiles 0,1 + V + one Q chain + 4 kb of
        # attention. Own tokens are chunk 0 (rotated layout).
        po0 = apsum.tile([65, 1024], f32, tag="po", name="po0")

        pass_K(0, 0, xf0)
        q_chain(0)
        pass_K(1, 0, xf0)
        pass_V(0, xf0)
        attn_kb(0, 0, 0, 4, po0)

        for ssl in (1, 2, 3):
            xf = xf_dma(ssl, f"xfa{ssl}")
            pass_K(0, ssl, xf)
            pass_K(1, ssl, xf)
            pass_V(ssl, xf)
            q_chain(ssl)
            attn_kb(0, 0, 4 * ssl, 4 * (ssl + 1), po0)
        attn_finish(0, po0)
        # rest of wk (tiles 2,3) for the later K passes
        nc.sync.dma_start(out=wk_sb[:, :, 2:4, :], in_=wk[:, :, 2:4, :])

        attn_pair(0, 1, interleave=((8, kpass(2, 0)),))
        attn_pair(0, 2, interleave=((8, kpass(2, 1)),))
        attn_pair(0, 3, interleave=((4, kpass(2, 2)), (10, qc(4))))

        attn_pair(1, 0, interleave=((2, lambda: recip_core(0)),
                                    (4, kpass(2, 3)), (8, qc(5)),
                                    (13, mul_fc(0))))
        attn_pair(1, 1, interleave=((4, kpass(3, 0)), (8, qc(6)),
                                    (13, mul_fc(1))))
        attn_pair(1, 2, interleave=((4, kpass(3, 1)), (8, qc(7)),
                                    (13, mul_fc(2))))
        attn_pair(1, 3, interleave=((4, kpass(3, 2)), (8, qc(8)),
                                    (13, mul_fc(3))))

        attn_pair(2, 0, interleave=((2, lambda: recip_core(1)),
                                    (4, kpass(3, 3)), (8, qc(9)),
                                    (13, mul_fc(4))))
        attn_pair(2, 1, interleave=((6, qc(10)), (13, mul_fc(5))))
        attn_pair(2, 2, interleave=((6, qc(11)), (13, mul_fc(6))))
        attn_pair(2, 3, interleave=((6, qc(12)), (13, mul_fc(7))))

        attn_pair(3, 0, interleave=((2, lambda: recip_core(2)),
                                    (6, qc(13)), (13, mul_fc(8))))
        attn_pair(3, 1, interleave=((6, qc(14)), (13, mul_fc(9))))
        attn_pair(3, 2, interleave=((6, qc(15)), (13, mul_fc(10))))
        attn_pair(3, 3, interleave=((13, mul_fc(11)),))

        # wo ring buffer reuses the dead wv slot: 4 chunks of
        # [128, 2, 1024], 4-deep prefetch via subtile WAR
        wo_ring = consts.tile([128, 8, 1024], bf16, tag="wv")

        def wo_dma(k, half, fcg):
            j = 2 * (k % 4)
            nc.sync.dma_start(
                out=wo_ring[:, j:j + 2, :],
                in_=wo[:, 2 * fcg:2 * fcg + 2, 1024 * half:1024 * (half + 1)])
            return wo_ring[:, j:j + 2, :]

        wo_pre = [wo_dma(0, 0, 0), wo_dma(1, 0, 1)]

        # warm-keepers bridge the final reciprocal chain (PE HAM clock)
        for i in range(8):
            sdum = psum.tile([128, 1024], f32, tag="s", name=f"sdum{i}")
            nc.tensor.matmul(
                sdum[0:65, 0:512], lhsT=vaug_sb[:, 0, 0:65],
                rhs=qt_sb[:, 0, :], start=True, stop=True,
            )
        recip_core(3)
        for fc in (12, 13, 14, 15):
            mul_fc(fc)()

        # ---- output projection: out[tok, D] = o_norm @ wo.T
        # 8 held accumulators: psum 2 tiles -> 4 halves, opsum 2, apsum
        # [128,1024] -> 2 halves; wo streamed in [128, 2fc, 1024] chunks.
        for half in range(2):
            accs = []
            s_ts = []
            for i in range(2):
                s_t = psum.tile([128, 1024], f32, tag="s", name=f"oas{half}_{i}")
                s_ts.append(s_t)
                accs.append(s_t[:, 0:512])
                accs.append(s_t[:, 512:1024])
            o_ts = [opsum.tile([128, 512], f32, tag="o", name=f"oao{half}_{i}")
                    for i in range(2)]
            accs.extend(o_ts)
            po_t = apsum.tile([128, 1024], f32, tag="po", name=f"oap{half}")
            accs.append(po_t[:, 0:512])
            accs.append(po_t[:, 512:1024])
            # accs[tb2*2+dc2] covers out[tb2*128:+128, 1024*half+512*dc2:+512]
            for fcg in range(8):
                k = half * 8 + fcg
                if k < 2:
                    wo_ch = wo_pre[k]
                else:
                    wo_ch = wo_dma(k, half, fcg)
                for fl in range(2):
                    fc = 2 * fcg + fl
                    for tb2 in range(4):
                        tsl = slice(tb2 * 128, (tb2 + 1) * 128)
                        for dc2 in range(2):
                            nc.tensor.matmul(
                                accs[tb2 * 2 + dc2],
                                lhsT=ot_sb[:, fc, tsl],
                                rhs=wo_ch[:, fl, 512 * dc2:512 * (dc2 + 1)],
                                start=(fc == 0), stop=(fc == DC - 1),
                            )
            # evacuate: merged copies + 1024-wide DMAs (gpsimd queue)
            for tb2, src_t in ((0, s_ts[0][:]), (1, s_ts[1][:]), (3, po_t[:])):
                osb = outw.tile([128, 1024], f32, tag="osb",
                                name=f"ob{half}_{tb2}")
                nc.vector.tensor_copy(osb, src_t)
                nc.gpsimd.dma_start(
                    out=out[tb2 * 128:(tb2 + 1) * 128,
                            1024 * half:1024 * (half + 1)],
                    in_=osb)
            osb = outw.tile([128, 1024], f32, tag="osb", name=f"ob{half}_2")
            nc.vector.tensor_copy(osb[:, 0:512], o_ts[0])
            nc.vector.tensor_copy(osb[:, 512:1024], o_ts[1])
            nc.gpsimd.dma_start(
                out=out[256:384, 1024 * half:1024 * (half + 1)], in_=osb)

    nc.compile()
    return nc


def _prep_shared(freqs_cos, freqs_sin, wqkv, wo):
    """Weight/table prep shared by all cores (token rotation applied later)."""
    cs = np.asarray(freqs_cos)[:, 0, :]  # [S, 64] (already repeat-2 layout)
    sn = np.asarray(freqs_sin)[:, 0, :]
    cos_h = np.empty((128, S), np.float32)
    sin_h = np.empty((128, S), np.float32)
    for p in range(128):
        cos_h[p] = cs[:, p % 64]
        sin_h[p] = sn[:, p % 64] * (-1.0 if p % 2 == 0 else 1.0)

    # Q rows permuted: fc = 4t+r -> [head 8t+r | head 8t+4+r]
    qrows = []
    for t in range(4):
        for r in range(4):
            for h in (8 * t + r, 8 * t + 4 + r):
                qrows.extend(range(h * HD, (h + 1) * HD))
    wq_t = np.ascontiguousarray(wqkv[qrows, :].T)  # [D, 2048]
    wq_h = np.ascontiguousarray(
        wq_t.reshape(DC, 128, DC, 128).transpose(1, 2, 0, 3)).astype(BF16)

    # K rows: tile t holds groups (2t | 2t+1)
    krows = []
    for t in range(4):
        for g in (2 * t, 2 * t + 1):
            krows.extend(range(H * HD + g * HD, H * HD + (g + 1) * HD))
    wk_t = np.ascontiguousarray(wqkv[krows, :].T)  # [D, 512]
    wk_h = np.ascontiguousarray(
        wk_t.reshape(DC, 128, 4, 128).transpose(1, 0, 2, 3)).astype(BF16)

    # V rows natural group order (cols t*128 : A 64 | B 64)
    vrows = list(range((H + G) * HD, (H + 2 * G) * HD))
    wv_t = np.ascontiguousarray(wqkv[vrows, :].T)  # [D, 512]
    wv_h = np.ascontiguousarray(
        wv_t.reshape(DC, 128, 512).transpose(1, 0, 2)).astype(BF16)

    # wo rhs: wo_h[p, fc, dcol] = wo[dcol, feat(fc, p)]
    feat = np.empty(D, np.int64)
    for fc in range(DC):
        t, r = divmod(fc, 4)
        for p in range(128):
            h = 8 * t + r + (4 if p >= 64 else 0)
            feat[fc * 128 + p] = h * HD + (p % 64)
    wo_h = np.ascontiguousarray(
        np.asarray(wo)[:, feat].T.reshape(DC, 128, D).transpose(1, 0, 2)
    ).astype(BF16)
    return cos_h, sin_h, wq_h, wk_h, wv_h, wo_h


def _prep_inputs(x, freqs_cos, freqs_sin, wqkv, wo):
    cos_h, sin_h, wq_h, wk_h, wv_h, wo_h = _prep_shared(
        freqs_cos, freqs_sin, wqkv, wo)
    x = np.asarray(x)
    ins = []
    for c in range(N_CORES):
        b, t4 = divmod(c, 4)
        q0 = t4 * TOK
        rot = (np.arange(S) + q0) % S  # own tokens land at cols 0:512
        xt_h = np.ascontiguousarray(
            x[b].T[:, rot].reshape(DC, 128, S).transpose(1, 0, 2)).astype(BF16)
        ins.append({
            "xt": xt_h,
            "wq": wq_h, "wk": wk_h, "wv": wv_h, "wo": wo_h,
            "cosr": np.ascontiguousarray(cos_h[:, rot]).astype(BF16),
            "sinr": np.ascontiguousarray(sin_h[:, rot]).astype(BF16),
        })
    return ins


TRACE = False


def kernel(x, freqs_cos, freqs_sin, wqkv, wo):
    if "nc" not in _CACHE:
        _CACHE["nc"] = _build()
    nc = _CACHE["nc"]
    ins = _prep_inputs(x, freqs_cos, freqs_sin, wqkv, wo)
    res = run_bass_kernel_spmd(nc, ins, list(range(N_CORES)), trace=TRACE)
    _CACHE["res"] = res
    out = np.empty((B, S, D), np.float32)
    for c in range(N_CORES):
        b, t4 = divmod(c, 4)
        out[b, t4 * TOK:(t4 + 1) * TOK, :] = res.results[c]["out"]
    return out


if __name__ == "__main__":
    rng = np.random.default_rng(0)
    x = rng.normal(size=(B, S, D)).astype(np.float32)
    fc_ = rng.random(size=(S, 1, HD)).astype(np.float32)
    fs_ = rng.random(size=(S, 1, HD)).astype(np.float32)
    wq_ = rng.normal(size=(3072, D)).astype(np.float32) * 0.02
    wo_ = rng.normal(size=(D, D)).astype(np.float32) * 0.02
    o = kernel(x, fc_, fs_, wq_, wo_)
    print(o.shape, o.dtype)
